# revision 9
# baseline (speedup 1.0000x reference)
"""Trainium2 Bass kernel for a 3-view GCN + attention fusion + autoencoder.

Row-parallel over 8 NeuronCores: core c owns node rows [c*1024, (c+1)*1024).
Adjacency blocks are fed pre-transposed (A_c^T, bf16) so they serve directly as
matmul operands. Per view v with features X [N, D], weights W1 [D, H], W2 [H, 256]:

  stage1: XW = X @ W1          (full rows, replicated on every core; lhsT = X^T)
  stage2: Z^T = relu((A_c @ XW)^T) = relu(XW^T @ A_c^T)   [H, 1024]
  stage3: ZW_c = Z @ W2        [1024, 256]
  AllGather(ZW) -> ZW_full [8192, 256]
  stage4: E_v = A_c @ ZW_full  [1024, 256]

Attention over the 3 views and the 8-layer autoencoder run row-locally; the
autoencoder runs in transposed orientation (features on partitions) so each
Linear's weight matrix [din, dout] is its lhsT as-is and bias+relu fuse into
one ACT op.  Heavy matmuls are bf16 with fp32 PSUM accumulation.
"""

import numpy as np
import ml_dtypes

import concourse.bacc as bacc
import concourse.mybir as mybir
import concourse.tile as tile
from concourse.bass_utils import run_bass_kernel_spmd
from concourse.masks import make_identity

BF16 = ml_dtypes.bfloat16
F32 = mybir.dt.float32
BF = mybir.dt.bfloat16

NCORES = 8
N = 8192
R = N // NCORES          # 1024 rows per core
KT = N // 128            # 64 k-blocks over nodes
DOUT = 256
ATT_H = 16

# (name, D, H) per view
VIEWS = [("v0", 512, 256), ("v1", 512, 256), ("vc", 1024, 512)]

# (wname, bname, din, dout, relu)
AE_LAYERS = [
    ("ew1", "eb1", 256, 500, True),
    ("ew2", "eb2", 500, 500, True),
    ("ew3", "eb3", 500, 2000, True),
    ("zw", "zb", 2000, 10, False),
    ("dw1", "db1", 10, 2000, True),
    ("dw2", "db2", 2000, 500, True),
    ("dw3", "db3", 500, 500, True),
    ("xw", "xb", 500, 256, False),
]

_CACHE = {}


def _ceil_div(a, b):
    return (a + b - 1) // b


def _build_nc():
    nc = bacc.Bacc("TRN2", target_bir_lowering=False, debug=False,
                   num_devices=NCORES)
    AF = mybir.ActivationFunctionType

    # ---------------- DRAM I/O ----------------
    aT = {}
    xT = {}
    w1_in = {}
    w2_in = {}
    for v, D, H in VIEWS:
        aT[v] = nc.dram_tensor(f"aT_{v}", [N, R], BF, kind="ExternalInput")
        xT[v] = nc.dram_tensor(f"xT_{v}", [D, N], BF, kind="ExternalInput")
        w1_in[v] = nc.dram_tensor(f"w1_{v}", [D, H], BF, kind="ExternalInput")
        w2_in[v] = nc.dram_tensor(f"w2_{v}", [H, DOUT], BF, kind="ExternalInput")

    # views stacked at 32-partition stride (PE out base partition must be 0/32/64)
    attw1_in = nc.dram_tensor("attw1", [DOUT, ATT_H], F32, kind="ExternalInput")
    attb1_in = nc.dram_tensor("attb1", [96, 1], F32, kind="ExternalInput")
    attw2_in = nc.dram_tensor("attw2", [96, 3], F32, kind="ExternalInput")

    ae_w_in = {}
    ae_b_in = {}
    for wn, bn, din, dout, _ in AE_LAYERS:
        ae_w_in[wn] = nc.dram_tensor(f"ae_{wn}", [din, dout], BF, kind="ExternalInput")
        ae_b_in[bn] = nc.dram_tensor(f"ae_{bn}", [dout, 1], F32, kind="ExternalInput")

    e_out = {v: nc.dram_tensor(f"e_{v}", [R, DOUT], F32, kind="ExternalOutput")
             for v, _, _ in VIEWS}
    emb_out = nc.dram_tensor("emb", [R, DOUT], F32, kind="ExternalOutput")
    xbar_out = nc.dram_tensor("xbar", [R, DOUT], F32, kind="ExternalOutput")
    z_out = nc.dram_tensor("z", [R, 10], F32, kind="ExternalOutput")
    beta_out = nc.dram_tensor("beta", [R, 3], F32, kind="ExternalOutput")

    with tile.TileContext(nc) as tc:
        with (
            tc.tile_pool(name="const", bufs=1) as const,
            tc.tile_pool(name="persist", bufs=1) as persist,
            tc.tile_pool(name="dram", bufs=1, space="DRAM") as dram,
        ):
            ident = const.tile([128, 128], F32, name="ident")
            make_identity(nc, ident)

            # GCN weights resident in SBUF (bf16)
            w1sb = {}
            w2sb = {}
            for v, D, H in VIEWS:
                dt_, ht_ = D // 128, H // 128
                t1 = const.tile([128, dt_, H], BF, name=f"w1sb_{v}")
                nc.sync.dma_start(t1[:], w1_in[v].rearrange("(a p) d -> p a d", p=128))
                w1sb[v] = t1
                t2 = const.tile([128, ht_, DOUT], BF, name=f"w2sb_{v}")
                nc.sync.dma_start(t2[:], w2_in[v].rearrange("(a p) d -> p a d", p=128))
                w2sb[v] = t2

            attw1sb = const.tile([128, 2, ATT_H], F32, name="attw1sb")
            nc.sync.dma_start(attw1sb[:], attw1_in.rearrange("(a p) d -> p a d", p=128))
            attb1sb = const.tile([96, 1], F32, name="attb1sb")
            nc.sync.dma_start(attb1sb[:], attb1_in[:])
            attw2sb = const.tile([96, 3], F32, name="attw2sb")
            nc.sync.dma_start(attw2sb[:], attw2_in[:])

            # AE biases resident (f32, [128, mt] per layer)
            ae_bsb = {}
            for wn, bn, din, dout, _ in AE_LAYERS:
                mt = _ceil_div(dout, 128)
                bt = const.tile([128, mt], F32, name=f"bsb_{bn}")
                for m in range(mt):
                    mp = min(128, dout - m * 128)
                    nc.sync.dma_start(bt[:mp, m:m + 1],
                                      ae_b_in[bn][m * 128:m * 128 + mp, :])
                ae_bsb[bn] = bt

            # E tiles for the 3 views, kept for attention: [128, v, m8, DOUT] f32
            ev = persist.tile([128, 3, 8, DOUT], F32, name="ev")

            # collective buffers
            zwin = {}
            zwfull = {}
            for v, _, _ in VIEWS:
                zwin[v] = dram.tile([R, DOUT], BF, name=f"zwin_{v}")
                zwfull[v] = dram.tile([N, DOUT], BF, addr_space="Shared",
                                      name=f"zwfull_{v}")

            # ---------------- Phase A: per view stages 1-3 + AllGather ----------
            for v, D, H in VIEWS:
                dt_, ht_ = D // 128, H // 128
                with tc.tile_pool(name=f"xwp_{v}", bufs=1) as xwp:
                    xw = xwp.tile([128, KT, H], BF, name=f"xw_{v}")

                    # stage1: XW[k] = (X @ W1) row-block k, all k
                    with (
                        tc.tile_pool(name=f"s1x_{v}", bufs=2) as s1x,
                        tc.tile_pool(name=f"s1p_{v}", bufs=2, space="PSUM") as s1p,
                    ):
                        CH = 8  # 8 column chunks of 1024 nodes
                        for ch in range(CH):
                            xts = []
                            for kd in range(dt_):
                                t = s1x.tile([128, 1024], BF, tag=f"xt{kd}",
                                             name=f"xt_{v}_{kd}")
                                nc.sync.dma_start(
                                    t[:], xT[v][kd * 128:(kd + 1) * 128,
                                                ch * 1024:(ch + 1) * 1024])
                                xts.append(t)
                            for kk in range(8):
                                k = ch * 8 + kk
                                p1 = s1p.tile([128, H], F32, tag="p1",
                                              name=f"p1_{v}_{k}")
                                for kd in range(dt_):
                                    nc.tensor.matmul(
                                        p1[:],
                                        xts[kd][:, kk * 128:(kk + 1) * 128],
                                        w1sb[v][:, kd, :],
                                        start=(kd == 0), stop=(kd == dt_ - 1))
                                nc.vector.tensor_copy(xw[:, k, :], p1[:])

                    # stage2: Z^T = relu(XW^T @ A_c^T)  [H, 1024]
                    with tc.tile_pool(name=f"ztp_{v}", bufs=1) as ztp:
                        zt = [ztp.tile([128, R], BF, name=f"zt_{v}_{mh}")
                              for mh in range(ht_)]
                        with (
                            tc.tile_pool(name=f"s2a_{v}", bufs=3) as s2a,
                            tc.tile_pool(name=f"s2p_{v}", bufs=1, space="PSUM") as s2p,
                        ):
                            p2 = [s2p.tile([128, R], F32, name=f"p2_{v}_{mh}")
                                  for mh in range(ht_)]
                            for k in range(KT):
                                at = s2a.tile([128, R], BF, tag="a",
                                              name=f"a2_{v}_{k}")
                                nc.sync.dma_start(at[:],
                                                  aT[v][k * 128:(k + 1) * 128, :])
                                for mh in range(ht_):
                                    for nh in range(2):
                                        nc.tensor.matmul(
                                            p2[mh][:, nh * 512:(nh + 1) * 512],
                                            xw[:, k, mh * 128:(mh + 1) * 128],
                                            at[:, nh * 512:(nh + 1) * 512],
                                            start=(k == 0), stop=(k == KT - 1))
                            for mh in range(ht_):
                                nc.scalar.activation(zt[mh][:], p2[mh][:],
                                                     AF.Relu)

                        # stage3: ZW = Z @ W2  [1024, 256] -> DRAM (bf16)
                        with (
                            tc.tile_pool(name=f"s3p_{v}", bufs=2, space="PSUM") as s3p,
                            tc.tile_pool(name=f"s3s_{v}", bufs=2) as s3s,
                        ):
                            for m8 in range(8):
                                p3 = s3p.tile([128, DOUT], F32, tag="p3",
                                              name=f"p3_{v}_{m8}")
                                for kh in range(ht_):
                                    nc.tensor.matmul(
                                        p3[:],
                                        zt[kh][:, m8 * 128:(m8 + 1) * 128],
                                        w2sb[v][:, kh, :],
                                        start=(kh == 0), stop=(kh == ht_ - 1))
                                zwt = s3s.tile([128, DOUT], BF, tag="zwt",
                                               name=f"zwt_{v}_{m8}")
                                nc.vector.tensor_copy(zwt[:], p3[:])
                                nc.sync.dma_start(
                                    zwin[v][m8 * 128:(m8 + 1) * 128, :], zwt[:])

                nc.gpsimd.collective_compute(
                    "AllGather",
                    mybir.AluOpType.bypass,
                    replica_groups=[list(range(NCORES))],
                    ins=[zwin[v].opt()],
                    outs=[zwfull[v].opt()],
                )

            # ---------------- Phase B: stage4 per view ----------------
            for vi, (v, D, H) in enumerate(VIEWS):
                with (
                    tc.tile_pool(name=f"s4a_{v}", bufs=3) as s4a,
                    tc.tile_pool(name=f"s4z_{v}", bufs=3) as s4z,
                    tc.tile_pool(name=f"s4p_{v}", bufs=1, space="PSUM") as s4p,
                ):
                    p4 = [s4p.tile([128, DOUT], F32, name=f"p4_{v}_{m8}")
                          for m8 in range(8)]
                    for k in range(KT):
                        at = s4a.tile([128, R], BF, tag="a4", name=f"a4_{v}_{k}")
                        nc.sync.dma_start(at[:], aT[v][k * 128:(k + 1) * 128, :])
                        zf = s4z.tile([128, DOUT], BF, tag="zf",
                                      name=f"zf_{v}_{k}")
                        nc.sync.dma_start(zf[:],
                                          zwfull[v][k * 128:(k + 1) * 128, :])
                        for m8 in range(8):
                            nc.tensor.matmul(
                                p4[m8][:],
                                at[:, m8 * 128:(m8 + 1) * 128],
                                zf[:],
                                start=(k == 0), stop=(k == KT - 1))
                    for m8 in range(8):
                        nc.vector.tensor_copy(ev[:, vi, m8, :], p4[m8][:])
                        nc.sync.dma_start(e_out[v][m8 * 128:(m8 + 1) * 128, :],
                                          ev[:, vi, m8, :])

            # ---------------- Phase C: attention over views ----------------
            embT = persist.tile([128, 2, R], BF, name="embT")
            with tc.tile_pool(name="att_sb", bufs=1) as att_sb:
                ssb = att_sb.tile([3, R], F32, name="ssb")
                with (
                    tc.tile_pool(name="attt_ps", bufs=2, space="PSUM") as attt_ps,
                    tc.tile_pool(name="att_ps1", bufs=1, space="PSUM") as att_ps1,
                ):
                    # E^T per view: [128, v, h, 1024] f32
                    evT = att_sb.tile([128, 3, 2, R], F32, name="evT")
                    for vi in range(3):
                        for h in range(2):
                            for m8 in range(8):
                                pt = attt_ps.tile([128, 128], F32, tag="pt",
                                                  name=f"pt_{vi}_{h}_{m8}")
                                nc.tensor.matmul(
                                    pt[:],
                                    ev[:, vi, m8, h * 128:(h + 1) * 128],
                                    ident[:], start=True, stop=True)
                                nc.vector.tensor_copy(
                                    evT[:, vi, h, m8 * 128:(m8 + 1) * 128],
                                    pt[:])

                    # T_stack [96, 1024] = tanh(W1a^T @ E^T + b1), stride 32
                    psT = att_ps1.tile([96, R], F32, name="psT", bufs=1)
                    nc.vector.memset(psT[:], 0.0)  # keep pad rows finite
                    for vi in range(3):
                        for kh in range(2):
                            for nh in range(2):
                                nc.tensor.matmul(
                                    psT[vi * 32:vi * 32 + ATT_H,
                                        nh * 512:(nh + 1) * 512],
                                    attw1sb[:, kh, :],
                                    evT[:, vi, kh, nh * 512:(nh + 1) * 512],
                                    start=(kh == 0), stop=(kh == 1))
                    tsb = att_sb.tile([96, R], F32, name="tsb")
                    nc.scalar.activation(tsb[:], psT[:], AF.Tanh,
                                         bias=attb1sb[:])

                    # S [3, 1024] = blockdiag(W2a)^T @ T_stack
                    psS = att_ps1.tile([3, R], F32, name="psS", bufs=1)
                    for nh in range(2):
                        nc.tensor.matmul(psS[:, nh * 512:(nh + 1) * 512],
                                         attw2sb[:],
                                         tsb[:, nh * 512:(nh + 1) * 512],
                                         start=True, stop=True)
                    nc.vector.tensor_copy(ssb[:], psS[:])

                # per node-block: softmax over views, emb, embT
                with (
                    tc.tile_pool(name="sm_sb", bufs=3) as sm_sb,
                    tc.tile_pool(name="sm_ps", bufs=3, space="PSUM") as sm_ps,
                ):
                    for m8 in range(8):
                        pb = sm_ps.tile([128, 3], F32, tag="pb", name=f"pb_{m8}")
                        nc.tensor.matmul(pb[:],
                                         ssb[:, m8 * 128:(m8 + 1) * 128],
                                         ident[:3, :3], start=True, stop=True)
                        sc = sm_sb.tile([128, 3], F32, tag="sc", name=f"sc_{m8}")
                        nc.scalar.activation(sc[:], pb[:], AF.Exp)
                        ssum = sm_sb.tile([128, 1], F32, tag="ssum",
                                          name=f"ssum_{m8}")
                        nc.vector.reduce_sum(ssum[:], sc[:],
                                             axis=mybir.AxisListType.X)
                        rinv = sm_sb.tile([128, 1], F32, tag="rinv",
                                          name=f"rinv_{m8}")
                        nc.vector.reciprocal(rinv[:], ssum[:])
                        beta = sm_sb.tile([128, 3], F32, tag="beta",
                                          name=f"beta_{m8}")
                        nc.vector.tensor_scalar_mul(beta[:], sc[:], rinv[:])
                        nc.sync.dma_start(beta_out[m8 * 128:(m8 + 1) * 128, :],
                                          beta[:])

                        embm = sm_sb.tile([128, DOUT], F32, tag="embm",
                                          name=f"embm_{m8}")
                        tmp = sm_sb.tile([128, DOUT], F32, tag="tmp",
                                         name=f"tmp_{m8}")
                        nc.vector.tensor_scalar_mul(embm[:], ev[:, 0, m8, :],
                                                    beta[:, 0:1])
                        nc.vector.tensor_scalar_mul(tmp[:], ev[:, 1, m8, :],
                                                    beta[:, 1:2])
                        nc.vector.tensor_add(embm[:], embm[:], tmp[:])
                        nc.vector.tensor_scalar_mul(tmp[:], ev[:, 2, m8, :],
                                                    beta[:, 2:3])
                        nc.vector.tensor_add(embm[:], embm[:], tmp[:])
                        nc.sync.dma_start(emb_out[m8 * 128:(m8 + 1) * 128, :],
                                          embm[:])
                        for h in range(2):
                            pe = sm_ps.tile([128, 128], F32, tag="pe",
                                            name=f"pe_{m8}_{h}")
                            nc.tensor.matmul(pe[:],
                                             embm[:, h * 128:(h + 1) * 128],
                                             ident[:], start=True, stop=True)
                            nc.vector.tensor_copy(
                                embT[:, h, m8 * 128:(m8 + 1) * 128], pe[:])

            # ---------------- Phase D: autoencoder (transposed chain) --------
            # current activation: list of (ap to [<=128, R] slice, valid_rows)
            x_tiles = [(embT[:, 0, :], 128), (embT[:, 1, :], 128)]
            with (
                tc.tile_pool(name="ae_w", bufs=1) as ae_wp,
                tc.tile_pool(name="ae_x", bufs=1) as ae_xp,
                tc.tile_pool(name="ae_ps", bufs=2, space="PSUM") as ae_ps,
                tc.tile_pool(name="ae_out", bufs=2) as ae_out_p,
                tc.tile_pool(name="ae_ops", bufs=2, space="PSUM") as ae_ops,
            ):
                zf32 = None
                xbT = None
                for li, (wn, bn, din, dout, relu) in enumerate(AE_LAYERS):
                    kt = _ceil_div(din, 128)
                    mt = _ceil_div(dout, 128)
                    wsb = ae_wp.tile([128, kt, dout], BF, tag=f"w{li % 2}",
                                     name=f"wsb_{wn}")
                    for k in range(kt):
                        kr = min(128, din - k * 128)
                        nc.sync.dma_start(wsb[:kr, k, :],
                                          ae_w_in[wn][k * 128:k * 128 + kr, :])
                    newx = ae_xp.tile([128, mt, R], BF, tag=f"x{li % 2}",
                                      name=f"x_{wn}")
                    if not relu:
                        fout = ae_xp.tile([128, mt, R], F32, name=f"xf_{wn}")
                    for m in range(mt):
                        mp = min(128, dout - m * 128)
                        pm = ae_ps.tile([128, R], F32, tag="aepm",
                                        name=f"pm_{wn}_{m}")
                        for k in range(kt):
                            kr = min(128, din - k * 128)
                            xa, _xr = x_tiles[k]
                            for nh in range(2):
                                nc.tensor.matmul(
                                    pm[:mp, nh * 512:(nh + 1) * 512],
                                    wsb[:kr, k, m * 128:m * 128 + mp],
                                    xa[:kr, nh * 512:(nh + 1) * 512],
                                    start=(k == 0), stop=(k == kt - 1))
                        if relu:
                            nc.scalar.activation(newx[:mp, m, :], pm[:mp, :],
                                                 AF.Relu,
                                                 bias=ae_bsb[bn][:mp, m:m + 1])
                        else:
                            nc.scalar.activation(fout[:mp, m, :], pm[:mp, :],
                                                 AF.Identity,
                                                 bias=ae_bsb[bn][:mp, m:m + 1])
                            nc.vector.tensor_copy(newx[:mp, m, :],
                                                  fout[:mp, m, :])
                    x_tiles = [(newx[:, k, :], min(128, dout - k * 128))
                               for k in range(mt)]
                    if wn == "zw":
                        zf32 = fout
                    elif wn == "xw":
                        xbT = fout

                # z output: transpose [10, 1024] -> [1024, 10]
                with tc.tile_pool(name="zo_sb", bufs=2) as zo_sb:
                    for m8 in range(8):
                        pz = ae_ops.tile([128, 10], F32, tag="pz",
                                         name=f"pz_{m8}")
                        nc.tensor.matmul(pz[:],
                                         zf32[:10, 0, m8 * 128:(m8 + 1) * 128],
                                         ident[:10, :10], start=True, stop=True)
                        zrow = zo_sb.tile([128, 10], F32, tag="zrow",
                                          name=f"zrow_{m8}")
                        nc.vector.tensor_copy(zrow[:], pz[:])
                        nc.sync.dma_start(z_out[m8 * 128:(m8 + 1) * 128, :],
                                          zrow[:])

                # x_bar output: transpose [256, 1024] -> [1024, 256]
                with tc.tile_pool(name="xb_sb", bufs=2) as xb_sb:
                    for m8 in range(8):
                        xrow = xb_sb.tile([128, 2, 128], F32, tag="xrow",
                                          name=f"xrow_{m8}")
                        for h in range(2):
                            px = ae_ops.tile([128, 128], F32, tag="px",
                                             name=f"px_{m8}_{h}")
                            nc.tensor.matmul(px[:],
                                             xbT[:, h, m8 * 128:(m8 + 1) * 128],
                                             ident[:], start=True, stop=True)
                            nc.vector.tensor_copy(xrow[:, h, :], px[:])
                        nc.sync.dma_start(xbar_out[m8 * 128:(m8 + 1) * 128, :],
                                          xrow[:])

    nc.compile()
    return nc


def _prep_in_maps(inputs):
    """Host-side sharding: per-core input dicts."""
    def bf(x):
        return np.ascontiguousarray(x).astype(BF16)

    # adjacency: cast to bf16 first (cheap), then transpose once
    aT_full = {}
    for v, adj in (("v0", inputs["adj0"]), ("v1", inputs["adj1"]),
                   ("vc", inputs["com_adj"])):
        aT_full[v] = np.ascontiguousarray(adj.astype(BF16).T)

    xT = {
        "v0": bf(inputs["f0"].T),
        "v1": bf(inputs["f1"].T),
        "vc": bf(inputs["com_fea"].T),
    }
    w1 = {"v0": bf(inputs["W1_0"]), "v1": bf(inputs["W1_1"]), "vc": bf(inputs["W1_c"])}
    w2 = {"v0": bf(inputs["W2_0"]), "v1": bf(inputs["W2_1"]), "vc": bf(inputs["W2_c"])}

    attw1 = np.ascontiguousarray(inputs["att_W1"], np.float32)
    attb1 = np.zeros((96, 1), np.float32)
    attw2 = np.zeros((96, 3), np.float32)
    for vi in range(3):
        attb1[vi * 32:vi * 32 + ATT_H, 0] = np.asarray(
            inputs["att_b1"], np.float32).reshape(-1)
        attw2[vi * 32:vi * 32 + ATT_H, vi] = np.asarray(
            inputs["att_W2"], np.float32).reshape(-1)

    ae_names = {
        "ew1": "enc1_W", "eb1": "enc1_b", "ew2": "enc2_W", "eb2": "enc2_b",
        "ew3": "enc3_W", "eb3": "enc3_b", "zw": "z_W", "zb": "z_b",
        "dw1": "dec1_W", "db1": "dec1_b", "dw2": "dec2_W", "db2": "dec2_b",
        "dw3": "dec3_W", "db3": "dec3_b", "xw": "xbar_W", "xb": "xbar_b",
    }

    shared = {}
    for v, _, _ in VIEWS:
        shared[f"xT_{v}"] = xT[v]
        shared[f"w1_{v}"] = w1[v]
        shared[f"w2_{v}"] = w2[v]
    shared["attw1"] = attw1
    shared["attb1"] = attb1
    shared["attw2"] = attw2
    for wn, bn, din, dout, _ in AE_LAYERS:
        shared[f"ae_{wn}"] = bf(inputs[ae_names[wn]])
        shared[f"ae_{bn}"] = np.asarray(
            inputs[ae_names[bn]], np.float32).reshape(-1, 1)

    in_maps = []
    for c in range(NCORES):
        m = dict(shared)
        for v, _, _ in VIEWS:
            m[f"aT_{v}"] = np.ascontiguousarray(
                aT_full[v][:, c * R:(c + 1) * R])
        in_maps.append(m)
    return in_maps


def run(inputs, trace=False, trace_kwargs=None):
    """Run on hardware; returns (outputs_tuple, BassKernelResults)."""
    if "nc" not in _CACHE:
        _CACHE["nc"] = _build_nc()
    nc = _CACHE["nc"]
    in_maps = _prep_in_maps(inputs)
    kw = {}
    if trace:
        kw = {"trace": True, "trace_kwargs": trace_kwargs or {}}
    res = run_bass_kernel_spmd(nc, in_maps, list(range(NCORES)), **kw)
    outs = res.results

    def gather(name):
        return np.concatenate([outs[c][name] for c in range(NCORES)], axis=0)

    e0 = gather("e_v0")
    e1 = gather("e_v1")
    ec = gather("e_vc")
    emb = gather("emb")
    x_bar = gather("xbar")
    z = gather("z")
    beta = gather("beta")[:, :, None]
    return (e0, e1, ec, emb, x_bar, z, beta), res


def kernel(**inputs):
    out, _ = run(inputs)
    return out


def time_hw(inputs, iters=10):
    """Estimate per-execution HW time: device-resident inputs, no donation,
    wall-clock over `iters` back-to-back dispatches."""
    import time
    import jax
    import jax.numpy as jnp
    from jax.experimental.shard_map import shard_map
    from jax.sharding import Mesh, PartitionSpec, NamedSharding
    from concourse import bass2jax
    from concourse.bass2jax import _bass_exec_p, partition_id_tensor
    import concourse.mybir as mybir_

    if "nc" not in _CACHE:
        _CACHE["nc"] = _build_nc()
    nc = _CACHE["nc"]
    bass2jax.install_neuronx_cc_hook()
    in_maps = _prep_in_maps(inputs)

    partition_name = nc.partition_id_tensor.name if nc.partition_id_tensor else None
    in_names, out_names, out_avals, zero_outs = [], [], [], []
    for alloc in nc.m.functions[0].allocations:
        if not isinstance(alloc, mybir_.MemoryLocationSet):
            continue
        name = alloc.memorylocations[0].name
        if alloc.kind == "ExternalInput":
            if name != partition_name:
                in_names.append(name)
        elif alloc.kind == "ExternalOutput":
            shape = tuple(alloc.tensor_shape)
            dtype = mybir_.dt.np(alloc.dtype)
            out_names.append(name)
            out_avals.append(jax.core.ShapedArray(shape, dtype))
            zero_outs.append(np.zeros(shape, dtype))
    n_params = len(in_names)
    all_in_names = list(in_names) + list(out_names)
    if partition_name is not None:
        all_in_names.append(partition_name)

    def _body(*args):
        operands = list(args)
        if partition_name is not None:
            operands.append(partition_id_tensor())
        outs = _bass_exec_p.bind(
            *operands,
            out_avals=tuple(out_avals),
            in_names=tuple(all_in_names),
            out_names=tuple(out_names),
            lowering_input_output_aliases=(),
            sim_require_finite=True,
            sim_require_nnan=True,
            nc=nc,
        )
        return tuple(outs)

    devices = jax.devices()[:NCORES]
    mesh = Mesh(np.asarray(devices), ("core",))
    spec = PartitionSpec("core")
    n_outs = len(out_names)
    sharded = jax.jit(
        shard_map(_body, mesh=mesh, in_specs=(spec,) * (n_params + n_outs),
                  out_specs=(spec,) * n_outs, check_rep=False),
        keep_unused=True,
    )
    shd = NamedSharding(mesh, spec)
    dev_in = [
        jax.device_put(
            np.concatenate([np.asarray(in_maps[c][nm]) for c in range(NCORES)], 0),
            shd)
        for nm in in_names
    ]
    dev_zero = [
        jax.device_put(np.zeros((NCORES * z.shape[0], *z.shape[1:]), z.dtype), shd)
        for z in zero_outs
    ]
    # warmup (compiles)
    out = sharded(*dev_in, *dev_zero)
    jax.block_until_ready(out)
    t0 = time.time()
    for _ in range(iters):
        out = sharded(*dev_in, *dev_zero)
    jax.block_until_ready(out)
    dt = (time.time() - t0) / iters
    # single-call latency
    lat = []
    for _ in range(3):
        t1 = time.time()
        out = sharded(*dev_in, *dev_zero)
        jax.block_until_ready(out)
        lat.append(time.time() - t1)
    return dt, min(lat)
